# revision 30
# baseline (speedup 1.0000x reference)
"""Conv3x3(8->64) + GroupNorm(16) + scale + MaxPool4 + clamp kernel for TRN2.

v2 layout (per core, S=16 samples):
  i4 [128=(g,ic,r), 4096] fp16: partition (g,ic,r) holds x[ic, 32g+r ...] rows,
    giving each of 4 PE row-strips (tile_position (32g,0)) its own data.
  conv: K=32 fp16 matmuls, N=512 (4 pairs x 128 w incl 2 pad cols), 4 strips
    concurrent; psum rounds [128, 2048] (4 banks), 4 rounds/sample.
  drain (ACT): Identity+bias fp32->fp16 with accum_out=sum; out layout per
    pair-row of 128: 4 w-quadrants of 32 (w = 4*w4 + r -> col 32r + w4) so all
    pool/Square DVE ops get step-1 inner dims (2x/4x modes).
  sumsq: per round TT(y*y in place, 2x) + tensor_scalar accum (4x); garbage
    cols {95,127} are summed (gsub) and zeroed first so stats stay exact.
  pools: lvl1w/lvl2w (DVE 2x), hpool+jfold+finalize on GPSIMD, jfold via DMA.
  stats: SEL matmul into a spare PSUM corner; ACT copies to SBUF; tiny chain.
"""

import numpy as np
import concourse.bass as bass
import concourse.tile as tile
from concourse import bacc, mybir
from contextlib import ExitStack

F32 = mybir.dt.float32
F16 = mybir.dt.float16
AL = mybir.AluOpType
AF = mybir.ActivationFunctionType
AX = mybir.AxisListType

EPS = 1e-5
NPIX = 4 * 126 * 126
XSAMP = 8 * 128 * 128   # elements per sample of x
F_I4 = 4096             # i4 free elements per partition
OSAMP = 64 * 31 * 31

# Tuning knobs
ACT_SQ_PAIRS = 1        # round-pairs (of 2) whose sumsq runs on ACT (else DVE)
FIN_ACT = True          # finalize via two ACT Relus (else DVE tensor_scalars)


def _ap(base, dims, offset):
    """Copy of AP `base` with raw [step,count] dims and element offset."""
    a = base.copy()
    a.ap = mybir.VecI64Pair([list(d) for d in dims])
    a.offset = offset
    return a


def _apf(base, free_dims, elem_offset):
    """SBUF AP: keep `base`'s partition dim, replace free dims, add offset."""
    a = base.copy()
    a.ap = mybir.VecI64Pair([list(base.ap[0])] + [list(d) for d in free_dims])
    a.offset = base.offset + elem_offset
    return a


class Pools:
    pass


def build_pools(ctx, tc):
    p = Pools()
    p.consts = ctx.enter_context(tc.tile_pool(name="consts", bufs=1))
    p.i4 = ctx.enter_context(tc.tile_pool(name="i4", bufs=2))
    p.ps = ctx.enter_context(tc.tile_pool(name="psc", bufs=2, space="PSUM"))
    p.y = ctx.enter_context(tc.tile_pool(name="ybuf", bufs=2))
    p.pool = ctx.enter_context(tc.tile_pool(name="pools", bufs=2))
    p.st = ctx.enter_context(tc.tile_pool(name="stats", bufs=2))
    return p


def load_consts(nc, p, w_in, sel_in, bias_in, ws_in, gb_in):
    c = Pools()
    c.wk = p.consts.tile([128, 384], F16, tag="w128")
    nc.sync.dma_start(c.wk[:], w_in[:])
    c.sel = p.consts.tile([128, 64], F32, tag="sel")
    nc.sync.dma_start(c.sel[:], sel_in[:])
    c.cbias = p.consts.tile([128, 1], F32, tag="cbias")
    nc.sync.dma_start(c.cbias[:], bias_in[:])
    c.ws = p.consts.tile([64, 1], F32, tag="ws")
    nc.sync.dma_start(c.ws[:], ws_in[:])
    c.gb = p.consts.tile([64, 1], F32, tag="gb")
    nc.sync.dma_start(c.gb[:], gb_in[:])
    c.negws = p.consts.tile([64, 1], F32, tag="negws")
    nc.vector.tensor_scalar(c.negws[:], c.ws[:], -1.0, None, op0=AL.mult)
    c.c1g = p.consts.tile([64, 1], F32, tag="c1g")
    nc.vector.tensor_scalar(c.c1g[:], c.gb[:], -1.0, 1.0, op0=AL.mult,
                            op1=AL.add)
    return c


def sample_body(nc, tc, p, c, x_in, y_out, n):
    # --- input DMA: i4[(g,ic,r), m*128 + w] = x[n, ic, 32g + r + m, w]
    i4 = p.i4.tile([128, F_I4], F16, tag="i4")
    base = i4[:]
    pstep = base.ap[0][0]
    for g in range(4):
        src = _ap(x_in, [[16384, 8], [128, 4], [1, F_I4]],
                  n * XSAMP + 4096 * g)
        nc.sync.dma_start(i4[32 * g:32 * g + 32, :], src)

    # ybf row R = 16*round + 4*g + pp (128 els: w-quadrants of 32, w=4*w4+r)
    ybf = p.y.tile([128, 8192], F16, tag="ybf")
    p1 = p.pool.tile([128, 4096], F16, tag="p1")
    sacc = p.st.tile([128, 12], F32, tag="sacc")
    # sacc cols: 0..7 drain sums, 8..9 garbage sums, 10..11 sumsq per cpair

    # row 63 (t15 pp3) is never drained; zero it so round-3 stats stay exact
    nc.vector.memset(_apf(ybf[:], [[1, 128]], 63 * 128), 0.0)

    # --- conv matmuls: rounds c=0..3; per round: [128,1536] tile (g0..g2,
    # bufs=2) + [128,512] g3 tile (bufs=1); stats matmul gets its own bank.
    for cp in range(2):
        tiles = []
        g3t = []
        for k in range(2):
            ps = p.ps.tile([128, 1536], F32, tag="ps")
            tiles.append(ps)
            ps3 = p.ps.tile([128, 512], F32, tag="psg3", bufs=1)
            g3t.append(ps3)
        for kw in range(3):
            for g in range(4):
                lhsT = c.wk[32 * g:32 * g + 32, 128 * kw:128 * kw + 128]
                for k, ci in ((0, 2 * cp), (1, 2 * cp + 1)):
                    t = 4 * g + ci
                    npp = 3 if t == 15 else 4
                    i4s = i4[32 * g:32 * g + 32, :]
                    rhs = _apf(i4s, [[256, npp], [1, 128]], 1024 * ci + kw)
                    if g < 3:
                        out = tiles[k][:, 512 * g: 512 * g + 128 * npp]
                    else:
                        out = g3t[k][:, 0: 128 * npp]
                    nc.tensor.matmul(out, lhsT, rhs,
                                     start=(kw == 0), stop=(kw == 2),
                                     tile_position=(32 * g, 0))

        # --- drains for the two rounds of this pair (g0-2 op + g3 op)
        for k, ci in ((0, 2 * cp), (1, 2 * cp + 1)):
            in_ap = _apf(tiles[k][:], [[512, 3], [1, 512]], 0)
            out_ap = _apf(ybf[:], [[128, 12], [1, 32], [32, 4]], 2048 * ci)
            nc.scalar.activation(out_ap, in_ap, AF.Identity,
                                 bias=c.cbias[:, 0:1], scale=1.0,
                                 accum_out=sacc[:, 2 * ci:2 * ci + 1])
            ntl = 384 if ci == 3 else 512
            in_ap = _apf(g3t[k][:], [[1, ntl]], 0)
            out_ap = _apf(ybf[:], [[128, ntl // 128], [1, 32], [32, 4]],
                          2048 * ci + 1536)
            nc.scalar.activation(out_ap, in_ap, AF.Identity,
                                 bias=c.cbias[:, 0:1], scale=1.0,
                                 accum_out=sacc[:, 2 * ci + 1:2 * ci + 2])

        # --- per-pair DVE work over rows 32*cp .. 32*cp+31
        rb = 4096 * cp
        # lvl1w: max over w-quadrant pairs (r0,r1) and (r2,r3)
        in0 = _apf(ybf[:], [[128, 32], [64, 2], [1, 32]], rb)
        in1 = _apf(ybf[:], [[128, 32], [64, 2], [1, 32]], rb + 32)
        o1 = _apf(p1[:], [[64, 32], [32, 2], [1, 32]], 2048 * cp)
        nc.vector.tensor_tensor(o1, in0, in1, op=AL.max)

        # garbage cols {95,127}: sum (for sum correction), then zero
        gap = _apf(ybf[:], [[128, 32], [32, 2]], rb + 95)
        nc.vector.reduce_sum(sacc[:, 8 + cp:9 + cp], gap, axis=AX.XY)
        nc.vector.memset(gap, 0.0)

        # sumsq of this pair's rows (in-place square then accum)
        yv = _apf(ybf[:], [[128, 32], [1, 128]], rb)
        if cp < ACT_SQ_PAIRS:
            nc.scalar.activation(yv, yv, AF.Square,
                                 accum_out=sacc[:, 10 + cp:11 + cp])
        else:
            nc.vector.tensor_tensor(yv, yv, yv, op=AL.mult)
            nc.vector.tensor_scalar(yv, yv, 1.0, 0.0, op0=AL.mult,
                                    op1=AL.add,
                                    accum_out=sacc[:, 10 + cp:11 + cp])

    # --- pool pyramid tail (rows stay in R = (round,g,pp) order)
    p2 = p.pool.tile([128, 2048], F16, tag="p2")
    in0 = _apf(p1[:], [[64, 64], [1, 32]], 0)
    in1 = _apf(p1[:], [[64, 64], [1, 32]], 32)
    o2 = _apf(p2[:], [[32, 64], [1, 32]], 0)
    nc.vector.tensor_tensor(o2, in0, in1, op=AL.max)

    p3 = p.pool.tile([128, 1024], F16, tag="p3")
    in0 = _apf(p2[:], [[64, 32], [1, 32]], 0)
    in1 = _apf(p2[:], [[64, 32], [1, 32]], 32)
    o3 = _apf(p3[:], [[32, 32], [1, 32]], 0)
    nc.vector.tensor_tensor(o3, in0, in1, op=AL.max)

    p3b = p.pool.tile([64, 1024], F16, tag="p3b")
    nc.sync.dma_start(p3b[:], p3[64:128, :])
    p4 = p.pool.tile([64, 1024], F16, tag="p4")
    nc.vector.tensor_tensor(p4[:], p3[0:64, :], p3b[:], op=AL.max)

    # --- stats
    stat2 = p.st.tile([128, 2], F32, tag="stat2")
    gtot = p.st.tile([128, 1], F32, tag="gtot")
    nc.vector.reduce_sum(stat2[:, 0:1], sacc[:, 0:8], axis=AX.X)
    nc.vector.reduce_sum(gtot[:], sacc[:, 8:10], axis=AX.X)
    nc.vector.tensor_tensor(stat2[:, 0:1], stat2[:, 0:1], gtot[:],
                            op=AL.subtract)
    nc.vector.reduce_sum(stat2[:, 1:2], sacc[:, 10:12], axis=AX.X)

    gs = p.ps.tile([64, 2], F32, tag="gsum", bufs=1)
    gsum = gs[:]
    nc.tensor.matmul(gsum, c.sel[:], stat2[:], start=True, stop=True,
                     skip_group_check=True)
    mv = p.st.tile([64, 2], F32, tag="mv")
    nc.vector.tensor_scalar(mv[:], gsum, 1.0 / NPIX, None, op0=AL.mult)

    msq = p.st.tile([64, 1], F32, tag="msq")
    nc.vector.tensor_tensor(msq[:], mv[:, 0:1], mv[:, 0:1], op=AL.mult)
    veps = p.st.tile([64, 1], F32, tag="veps")
    nc.vector.scalar_tensor_tensor(veps[:], mv[:, 1:2], EPS, msq[:],
                                   op0=AL.add, op1=AL.subtract)
    rv = p.st.tile([64, 1], F32, tag="rv")
    nc.vector.reciprocal(rv[:], veps[:])
    istd = p.st.tile([64, 1], F32, tag="istd")
    nc.scalar.activation(istd[:], rv[:], AF.Sqrt)

    # --- finalize: clamp(A*p + B, 0, 1) -> fp32 in output q-order, DMA out
    # p4 row k = 8c + 2g + u maps to output q = 8g + 2c + u
    fin = p.pool.tile([64, 1024], F16, tag="fin")
    outb = p.pool.tile([64, 992], F32, tag="outb")
    if FIN_ACT:
        # clamp(x,0,1) = Relu(1 - Relu(1 - x)); x = A*p + B folds into Relu#1
        nega = p.st.tile([64, 1], F32, tag="nega")
        nc.vector.tensor_tensor(nega[:], c.negws[:], istd[:], op=AL.mult)
        muap = p.st.tile([64, 1], F32, tag="muap")
        nc.vector.tensor_tensor(muap[:], mv[:, 0:1], nega[:], op=AL.mult)
        b1ap = p.st.tile([64, 1], F32, tag="b1ap")
        nc.vector.tensor_tensor(b1ap[:], c.c1g[:], muap[:], op=AL.subtract)
        nc.scalar.activation(fin[:], p4[:], AF.Relu, bias=b1ap[:, 0:1],
                             scale=nega[:, 0:1])
        for u in range(2):
            src = _apf(fin[:], [[256, 4], [64, 4], [1, 31]], 32 * u)
            dst = _apf(outb[:], [[62, 4], [248, 4], [1, 31]], 31 * u)
            nc.scalar.activation(dst, src, AF.Relu, bias=1.0, scale=-1.0)
    else:
        aap = p.st.tile([64, 1], F32, tag="aap")
        nc.vector.tensor_tensor(aap[:], c.ws[:], istd[:], op=AL.mult)
        mua = p.st.tile([64, 1], F32, tag="mua")
        nc.vector.tensor_tensor(mua[:], mv[:, 0:1], aap[:], op=AL.mult)
        bap = p.st.tile([64, 1], F32, tag="bap")
        nc.vector.tensor_tensor(bap[:], c.gb[:], mua[:], op=AL.subtract)
        nc.vector.tensor_scalar(fin[:], p4[:], aap[:, 0:1], bap[:, 0:1],
                                op0=AL.mult, op1=AL.add)
        for u in range(2):
            src = _apf(fin[:], [[256, 4], [64, 4], [1, 31]], 32 * u)
            dst = _apf(outb[:], [[62, 4], [248, 4], [1, 31]], 31 * u)
            nc.vector.tensor_scalar(dst, src, 0.0, 1.0, op0=AL.max, op1=AL.min)

    dstp = _ap(y_out, [[961, 64], [1, 961]], n * OSAMP)
    nc.sync.dma_start(dstp, outb[:, 0:961])


def build_kernel_nc(S, n_cores=8, repeat=1, use_for_i=False):
    nc = bacc.Bacc("TRN2", target_bir_lowering=False, debug=False,
                   num_devices=n_cores)
    x_in = nc.dram_tensor("x", [S * XSAMP + F_I4], F16,
                          kind="ExternalInput").ap()
    w_in = nc.dram_tensor("w128", [128, 384], F16, kind="ExternalInput").ap()
    sel_in = nc.dram_tensor("sel", [128, 64], F32, kind="ExternalInput").ap()
    bias_in = nc.dram_tensor("cbias", [128, 1], F32, kind="ExternalInput").ap()
    ws_in = nc.dram_tensor("ws", [64, 1], F32, kind="ExternalInput").ap()
    gb_in = nc.dram_tensor("gb", [64, 1], F32, kind="ExternalInput").ap()
    y_out = nc.dram_tensor("y", [S, 64, 31, 31], F32, kind="ExternalOutput").ap()
    with tile.TileContext(nc) as tc:
        with ExitStack() as ctx:
            p = build_pools(ctx, tc)
            c = load_consts(nc, p, w_in, sel_in, bias_in, ws_in, gb_in)
            if use_for_i and repeat > 1:
                with tc.For_i(0, repeat, 1):
                    for n in range(S):
                        sample_body(nc, tc, p, c, x_in, y_out, n)
            else:
                for _ in range(repeat):
                    for n in range(S):
                        sample_body(nc, tc, p, c, x_in, y_out, n)
    nc.compile()
    return nc


def make_consts(conv_w, conv_b, gn_w, gn_b, scale):
    """Host-side constant assembly."""
    conv_w = np.asarray(conv_w, np.float32)
    w128 = np.zeros((128, 384), np.float32)
    oc = np.arange(64)
    for kw in range(3):
        for j in range(2):
            for ic in range(8):
                for kh in range(3):
                    w128[4 * ic + kh + j, 128 * kw + oc + 64 * j] = \
                        conv_w[oc, ic, kh, kw]
    w128[32:64] = w128[0:32]
    w128[64:96] = w128[0:32]
    w128[96:128] = w128[0:32]
    sel = np.zeros((128, 64), np.float32)
    for j in range(2):
        for o in range(64):
            sel[o + 64 * j, (o // 4) * 4: (o // 4) * 4 + 4] = 1.0
    cbias = np.tile(np.asarray(conv_b, np.float32).reshape(64, 1), (2, 1))
    ws = (np.asarray(gn_w, np.float32).reshape(64) *
          np.asarray(scale, np.float32).reshape(64)).reshape(64, 1)
    gb = (np.asarray(gn_b, np.float32).reshape(64) *
          np.asarray(scale, np.float32).reshape(64)).reshape(64, 1)
    return dict(w128=w128.astype(np.float16), sel=sel,
                cbias=cbias.astype(np.float32), ws=ws.astype(np.float32),
                gb=gb.astype(np.float32))


def make_x_shard(x, core):
    """fp16, flattened, padded shard of x for one core."""
    sh = np.asarray(x[core * S_PER_CORE:(core + 1) * S_PER_CORE],
                    dtype=np.float16)
    flat = np.zeros(S_PER_CORE * XSAMP + F_I4, np.float16)
    flat[:S_PER_CORE * XSAMP] = sh.ravel()
    return flat


# ---------------------------------------------------------------------------
# Harness entry point: full (unsharded) inputs -> full output.
# ---------------------------------------------------------------------------
N_CORES = 8
S_PER_CORE = 16
_NC_CACHE = {}


def _get_nc(repeat=1, use_for_i=False):
    key = (repeat, use_for_i)
    if key not in _NC_CACHE:
        _NC_CACHE[key] = build_kernel_nc(S_PER_CORE, n_cores=N_CORES,
                                         repeat=repeat, use_for_i=use_for_i)
    return _NC_CACHE[key]


def kernel(x, conv_w, conv_b, gn_w, gn_b, scale):
    from concourse.bass_utils import run_bass_kernel_spmd
    x = np.asarray(x)
    consts = make_consts(conv_w, conv_b, gn_w, gn_b, scale)
    nc = _get_nc()
    in_maps = []
    for c in range(N_CORES):
        m = dict(consts)
        m["x"] = make_x_shard(x, c)
        in_maps.append(m)
    res = run_bass_kernel_spmd(nc, in_maps, core_ids=list(range(N_CORES)))
    return np.concatenate([res.results[c]["y"] for c in range(N_CORES)],
                          axis=0)


# revision 31
# speedup vs baseline: 1.0554x; 1.0554x over previous
"""Conv3x3(8->64) + GroupNorm(16) + scale + MaxPool4 + clamp kernel for TRN2.

v2 layout (per core, S=16 samples):
  i4 [128=(g,ic,r), 4096] fp16: partition (g,ic,r) holds x[ic, 32g+r ...] rows,
    giving each of 4 PE row-strips (tile_position (32g,0)) its own data.
  conv: K=32 fp16 matmuls, N=512 (4 pairs x 128 w incl 2 pad cols), 4 strips
    concurrent; psum rounds [128, 2048] (4 banks), 4 rounds/sample.
  drain (ACT): Identity+bias fp32->fp16 with accum_out=sum; out layout per
    pair-row of 128: 4 w-quadrants of 32 (w = 4*w4 + r -> col 32r + w4) so all
    pool/Square DVE ops get step-1 inner dims (2x/4x modes).
  sumsq: per round TT(y*y in place, 2x) + tensor_scalar accum (4x); garbage
    cols {95,127} are summed (gsub) and zeroed first so stats stay exact.
  pools: lvl1w/lvl2w (DVE 2x), hpool+jfold+finalize on GPSIMD, jfold via DMA.
  stats: SEL matmul into a spare PSUM corner; ACT copies to SBUF; tiny chain.
"""

import numpy as np
import concourse.bass as bass
import concourse.tile as tile
from concourse import bacc, mybir
from contextlib import ExitStack

F32 = mybir.dt.float32
F16 = mybir.dt.float16
AL = mybir.AluOpType
AF = mybir.ActivationFunctionType
AX = mybir.AxisListType

EPS = 1e-5
NPIX = 4 * 126 * 126
XSAMP = 8 * 128 * 128   # elements per sample of x
F_I4 = 4096             # i4 free elements per partition
OSAMP = 64 * 31 * 31

# Tuning knobs
ACT_SQ_PAIRS = 1        # round-pairs (of 2) whose sumsq runs on ACT (else DVE)
FIN_ACT = True          # finalize via two ACT Relus (else DVE tensor_scalars)


def _ap(base, dims, offset):
    """Copy of AP `base` with raw [step,count] dims and element offset."""
    a = base.copy()
    a.ap = mybir.VecI64Pair([list(d) for d in dims])
    a.offset = offset
    return a


def _apf(base, free_dims, elem_offset):
    """SBUF AP: keep `base`'s partition dim, replace free dims, add offset."""
    a = base.copy()
    a.ap = mybir.VecI64Pair([list(base.ap[0])] + [list(d) for d in free_dims])
    a.offset = base.offset + elem_offset
    return a


class Pools:
    pass


def build_pools(ctx, tc):
    p = Pools()
    p.consts = ctx.enter_context(tc.tile_pool(name="consts", bufs=1))
    p.i4 = ctx.enter_context(tc.tile_pool(name="i4", bufs=2))
    p.ps = ctx.enter_context(tc.tile_pool(name="psc", bufs=2, space="PSUM"))
    p.y = ctx.enter_context(tc.tile_pool(name="ybuf", bufs=2))
    p.pool = ctx.enter_context(tc.tile_pool(name="pools", bufs=2))
    p.st = ctx.enter_context(tc.tile_pool(name="stats", bufs=2))
    return p


def load_consts(nc, p, w_in, sel_in, bias_in, ws_in, gb_in):
    c = Pools()
    c.wk = p.consts.tile([128, 384], F16, tag="w128")
    nc.sync.dma_start(c.wk[:], w_in[:])
    c.sel = p.consts.tile([128, 64], F32, tag="sel")
    nc.sync.dma_start(c.sel[:], sel_in[:])
    c.cbias = p.consts.tile([128, 1], F32, tag="cbias")
    nc.sync.dma_start(c.cbias[:], bias_in[:])
    c.ws = p.consts.tile([64, 1], F32, tag="ws")
    nc.sync.dma_start(c.ws[:], ws_in[:])
    c.gb = p.consts.tile([64, 1], F32, tag="gb")
    nc.sync.dma_start(c.gb[:], gb_in[:])
    c.negws = p.consts.tile([64, 1], F32, tag="negws")
    nc.vector.tensor_scalar(c.negws[:], c.ws[:], -1.0, None, op0=AL.mult)
    c.c1g = p.consts.tile([64, 1], F32, tag="c1g")
    nc.vector.tensor_scalar(c.c1g[:], c.gb[:], -1.0, 1.0, op0=AL.mult,
                            op1=AL.add)
    return c


def stage_a(nc, p, c, x_in, st, n):
    """DMA in, matmuls, PSUM drains."""
    # --- input DMA: i4[(g,ic,r), m*128 + w] = x[n, ic, 32g + r + m, w]
    i4 = p.i4.tile([128, F_I4], F16, tag="i4")
    for g in range(4):
        src = _ap(x_in, [[16384, 8], [128, 4], [1, F_I4]],
                  n * XSAMP + 4096 * g)
        nc.sync.dma_start(i4[32 * g:32 * g + 32, :], src)

    # ybf row R = 16*round + 4*g + pp (128 els: w-quadrants of 32, w=4*w4+r)
    ybf = p.y.tile([128, 8192], F16, tag="ybf")
    sacc = p.st.tile([128, 12], F32, tag="sacc", bufs=3)
    # sacc cols: 0..7 drain sums, 8..9 garbage sums, 10..11 sumsq per cpair
    st[n] = Pools()
    st[n].ybf = ybf
    st[n].sacc = sacc

    # row 63 (t15 pp3) is never drained; zero it so round-3 stats stay exact
    nc.vector.memset(_apf(ybf[:], [[1, 128]], 63 * 128), 0.0)

    # --- conv matmuls: rounds c=0..3; per round: [128,1536] tile (g0..g2,
    # bufs=2) + [128,512] g3 tile (bufs=1); stats matmul gets its own bank.
    for cp in range(2):
        tiles = []
        g3t = []
        for k in range(2):
            ps = p.ps.tile([128, 1536], F32, tag="ps")
            tiles.append(ps)
            ps3 = p.ps.tile([128, 512], F32, tag="psg3", bufs=1)
            g3t.append(ps3)
        for kw in range(3):
            for g in range(4):
                lhsT = c.wk[32 * g:32 * g + 32, 128 * kw:128 * kw + 128]
                for k, ci in ((0, 2 * cp), (1, 2 * cp + 1)):
                    t = 4 * g + ci
                    npp = 3 if t == 15 else 4
                    i4s = i4[32 * g:32 * g + 32, :]
                    rhs = _apf(i4s, [[256, npp], [1, 128]], 1024 * ci + kw)
                    if g < 3:
                        out = tiles[k][:, 512 * g: 512 * g + 128 * npp]
                    else:
                        out = g3t[k][:, 0: 128 * npp]
                    nc.tensor.matmul(out, lhsT, rhs,
                                     start=(kw == 0), stop=(kw == 2),
                                     tile_position=(32 * g, 0))

        # --- drains for the two rounds of this pair (g0-2 op + g3 op)
        for k, ci in ((0, 2 * cp), (1, 2 * cp + 1)):
            in_ap = _apf(tiles[k][:], [[512, 3], [1, 512]], 0)
            out_ap = _apf(ybf[:], [[128, 12], [1, 32], [32, 4]], 2048 * ci)
            nc.scalar.activation(out_ap, in_ap, AF.Identity,
                                 bias=c.cbias[:, 0:1], scale=1.0,
                                 accum_out=sacc[:, 2 * ci:2 * ci + 1])
            ntl = 384 if ci == 3 else 512
            in_ap = _apf(g3t[k][:], [[1, ntl]], 0)
            out_ap = _apf(ybf[:], [[128, ntl // 128], [1, 32], [32, 4]],
                          2048 * ci + 1536)
            nc.scalar.activation(out_ap, in_ap, AF.Identity,
                                 bias=c.cbias[:, 0:1], scale=1.0,
                                 accum_out=sacc[:, 2 * ci + 1:2 * ci + 2])


def stage_b(nc, p, c, st, n):
    """Pool pyramid + sum-of-squares (DVE + optional ACT square)."""
    ybf = st[n].ybf
    sacc = st[n].sacc
    p1 = p.pool.tile([128, 4096], F16, tag="p1")
    for cp in range(2):
        # --- per-pair DVE work over rows 32*cp .. 32*cp+31
        rb = 4096 * cp
        # lvl1w: max over w-quadrant pairs (r0,r1) and (r2,r3)
        in0 = _apf(ybf[:], [[128, 32], [64, 2], [1, 32]], rb)
        in1 = _apf(ybf[:], [[128, 32], [64, 2], [1, 32]], rb + 32)
        o1 = _apf(p1[:], [[64, 32], [32, 2], [1, 32]], 2048 * cp)
        nc.vector.tensor_tensor(o1, in0, in1, op=AL.max)

        # garbage cols {95,127}: sum (for sum correction), then zero
        gap = _apf(ybf[:], [[128, 32], [32, 2]], rb + 95)
        nc.vector.reduce_sum(sacc[:, 8 + cp:9 + cp], gap, axis=AX.XY)
        nc.vector.memset(gap, 0.0)

        # sumsq of this pair's rows (in-place square then accum)
        yv = _apf(ybf[:], [[128, 32], [1, 128]], rb)
        if cp < ACT_SQ_PAIRS:
            nc.scalar.activation(yv, yv, AF.Square,
                                 accum_out=sacc[:, 10 + cp:11 + cp])
        else:
            nc.vector.tensor_tensor(yv, yv, yv, op=AL.mult)
            nc.vector.tensor_scalar(yv, yv, 1.0, 0.0, op0=AL.mult,
                                    op1=AL.add,
                                    accum_out=sacc[:, 10 + cp:11 + cp])

    # --- pool pyramid tail (rows stay in R = (round,g,pp) order)
    p2 = p.pool.tile([128, 2048], F16, tag="p2")
    in0 = _apf(p1[:], [[64, 64], [1, 32]], 0)
    in1 = _apf(p1[:], [[64, 64], [1, 32]], 32)
    o2 = _apf(p2[:], [[32, 64], [1, 32]], 0)
    nc.vector.tensor_tensor(o2, in0, in1, op=AL.max)

    p3 = p.pool.tile([128, 1024], F16, tag="p3")
    in0 = _apf(p2[:], [[64, 32], [1, 32]], 0)
    in1 = _apf(p2[:], [[64, 32], [1, 32]], 32)
    o3 = _apf(p3[:], [[32, 32], [1, 32]], 0)
    nc.vector.tensor_tensor(o3, in0, in1, op=AL.max)

    p3b = p.pool.tile([64, 1024], F16, tag="p3b")
    nc.sync.dma_start(p3b[:], p3[64:128, :])
    p4 = p.pool.tile([64, 1024], F16, tag="p4")
    nc.vector.tensor_tensor(p4[:], p3[0:64, :], p3b[:], op=AL.max)

    st[n].p4 = p4


def stage_c(nc, p, c, y_out, st, n):
    """Group stats, finalize, output DMA."""
    sacc = st[n].sacc
    p4 = st[n].p4
    # --- stats
    stat2 = p.st.tile([128, 2], F32, tag="stat2")
    gtot = p.st.tile([128, 1], F32, tag="gtot")
    nc.vector.reduce_sum(stat2[:, 0:1], sacc[:, 0:8], axis=AX.X)
    nc.vector.reduce_sum(gtot[:], sacc[:, 8:10], axis=AX.X)
    nc.vector.tensor_tensor(stat2[:, 0:1], stat2[:, 0:1], gtot[:],
                            op=AL.subtract)
    nc.vector.reduce_sum(stat2[:, 1:2], sacc[:, 10:12], axis=AX.X)

    gs = p.ps.tile([64, 2], F32, tag="gsum", bufs=1)
    gsum = gs[:]
    nc.tensor.matmul(gsum, c.sel[:], stat2[:], start=True, stop=True,
                     skip_group_check=True)
    mv = p.st.tile([64, 2], F32, tag="mv")
    nc.vector.tensor_scalar(mv[:], gsum, 1.0 / NPIX, None, op0=AL.mult)

    msq = p.st.tile([64, 1], F32, tag="msq")
    nc.vector.tensor_tensor(msq[:], mv[:, 0:1], mv[:, 0:1], op=AL.mult)
    veps = p.st.tile([64, 1], F32, tag="veps")
    nc.vector.scalar_tensor_tensor(veps[:], mv[:, 1:2], EPS, msq[:],
                                   op0=AL.add, op1=AL.subtract)
    rv = p.st.tile([64, 1], F32, tag="rv")
    nc.vector.reciprocal(rv[:], veps[:])
    istd = p.st.tile([64, 1], F32, tag="istd")
    nc.scalar.activation(istd[:], rv[:], AF.Sqrt)

    # --- finalize: clamp(A*p + B, 0, 1) -> fp32 in output q-order, DMA out
    # p4 row k = 8c + 2g + u maps to output q = 8g + 2c + u
    fin = p.pool.tile([64, 1024], F16, tag="fin")
    outb = p.pool.tile([64, 992], F32, tag="outb")
    if FIN_ACT:
        # clamp(x,0,1) = Relu(1 - Relu(1 - x)); x = A*p + B folds into Relu#1
        nega = p.st.tile([64, 1], F32, tag="nega")
        nc.vector.tensor_tensor(nega[:], c.negws[:], istd[:], op=AL.mult)
        muap = p.st.tile([64, 1], F32, tag="muap")
        nc.vector.tensor_tensor(muap[:], mv[:, 0:1], nega[:], op=AL.mult)
        b1ap = p.st.tile([64, 1], F32, tag="b1ap")
        nc.vector.tensor_tensor(b1ap[:], c.c1g[:], muap[:], op=AL.subtract)
        nc.scalar.activation(fin[:], p4[:], AF.Relu, bias=b1ap[:, 0:1],
                             scale=nega[:, 0:1])
        for u in range(2):
            src = _apf(fin[:], [[256, 4], [64, 4], [1, 31]], 32 * u)
            dst = _apf(outb[:], [[62, 4], [248, 4], [1, 31]], 31 * u)
            nc.scalar.activation(dst, src, AF.Relu, bias=1.0, scale=-1.0)
    else:
        aap = p.st.tile([64, 1], F32, tag="aap")
        nc.vector.tensor_tensor(aap[:], c.ws[:], istd[:], op=AL.mult)
        mua = p.st.tile([64, 1], F32, tag="mua")
        nc.vector.tensor_tensor(mua[:], mv[:, 0:1], aap[:], op=AL.mult)
        bap = p.st.tile([64, 1], F32, tag="bap")
        nc.vector.tensor_tensor(bap[:], c.gb[:], mua[:], op=AL.subtract)
        nc.vector.tensor_scalar(fin[:], p4[:], aap[:, 0:1], bap[:, 0:1],
                                op0=AL.mult, op1=AL.add)
        for u in range(2):
            src = _apf(fin[:], [[256, 4], [64, 4], [1, 31]], 32 * u)
            dst = _apf(outb[:], [[62, 4], [248, 4], [1, 31]], 31 * u)
            nc.vector.tensor_scalar(dst, src, 0.0, 1.0, op0=AL.max, op1=AL.min)

    dstp = _ap(y_out, [[961, 64], [1, 961]], n * OSAMP)
    nc.sync.dma_start(dstp, outb[:, 0:961])
    del st[n]


def build_kernel_nc(S, n_cores=8, repeat=1, use_for_i=False):
    nc = bacc.Bacc("TRN2", target_bir_lowering=False, debug=False,
                   num_devices=n_cores)
    x_in = nc.dram_tensor("x", [S * XSAMP + F_I4], F16,
                          kind="ExternalInput").ap()
    w_in = nc.dram_tensor("w128", [128, 384], F16, kind="ExternalInput").ap()
    sel_in = nc.dram_tensor("sel", [128, 64], F32, kind="ExternalInput").ap()
    bias_in = nc.dram_tensor("cbias", [128, 1], F32, kind="ExternalInput").ap()
    ws_in = nc.dram_tensor("ws", [64, 1], F32, kind="ExternalInput").ap()
    gb_in = nc.dram_tensor("gb", [64, 1], F32, kind="ExternalInput").ap()
    y_out = nc.dram_tensor("y", [S, 64, 31, 31], F32, kind="ExternalOutput").ap()
    with tile.TileContext(nc) as tc:
        with ExitStack() as ctx:
            p = build_pools(ctx, tc)
            c = load_consts(nc, p, w_in, sel_in, bias_in, ws_in, gb_in)
            def body():
                st = {}
                for it in range(S + 2):
                    if it < S:
                        stage_a(nc, p, c, x_in, st, it)
                    if 1 <= it <= S:
                        stage_b(nc, p, c, st, it - 1)
                    if it >= 2:
                        stage_c(nc, p, c, y_out, st, it - 2)

            if use_for_i and repeat > 1:
                with tc.For_i(0, repeat, 1):
                    body()
            else:
                for _ in range(repeat):
                    body()
    nc.compile()
    return nc


def make_consts(conv_w, conv_b, gn_w, gn_b, scale):
    """Host-side constant assembly."""
    conv_w = np.asarray(conv_w, np.float32)
    w128 = np.zeros((128, 384), np.float32)
    oc = np.arange(64)
    for kw in range(3):
        for j in range(2):
            for ic in range(8):
                for kh in range(3):
                    w128[4 * ic + kh + j, 128 * kw + oc + 64 * j] = \
                        conv_w[oc, ic, kh, kw]
    w128[32:64] = w128[0:32]
    w128[64:96] = w128[0:32]
    w128[96:128] = w128[0:32]
    sel = np.zeros((128, 64), np.float32)
    for j in range(2):
        for o in range(64):
            sel[o + 64 * j, (o // 4) * 4: (o // 4) * 4 + 4] = 1.0
    cbias = np.tile(np.asarray(conv_b, np.float32).reshape(64, 1), (2, 1))
    ws = (np.asarray(gn_w, np.float32).reshape(64) *
          np.asarray(scale, np.float32).reshape(64)).reshape(64, 1)
    gb = (np.asarray(gn_b, np.float32).reshape(64) *
          np.asarray(scale, np.float32).reshape(64)).reshape(64, 1)
    return dict(w128=w128.astype(np.float16), sel=sel,
                cbias=cbias.astype(np.float32), ws=ws.astype(np.float32),
                gb=gb.astype(np.float32))


def make_x_shard(x, core):
    """fp16, flattened, padded shard of x for one core."""
    sh = np.asarray(x[core * S_PER_CORE:(core + 1) * S_PER_CORE],
                    dtype=np.float16)
    flat = np.zeros(S_PER_CORE * XSAMP + F_I4, np.float16)
    flat[:S_PER_CORE * XSAMP] = sh.ravel()
    return flat


# ---------------------------------------------------------------------------
# Harness entry point: full (unsharded) inputs -> full output.
# ---------------------------------------------------------------------------
N_CORES = 8
S_PER_CORE = 16
_NC_CACHE = {}


def _get_nc(repeat=1, use_for_i=False):
    key = (repeat, use_for_i)
    if key not in _NC_CACHE:
        _NC_CACHE[key] = build_kernel_nc(S_PER_CORE, n_cores=N_CORES,
                                         repeat=repeat, use_for_i=use_for_i)
    return _NC_CACHE[key]


def kernel(x, conv_w, conv_b, gn_w, gn_b, scale):
    from concourse.bass_utils import run_bass_kernel_spmd
    x = np.asarray(x)
    consts = make_consts(conv_w, conv_b, gn_w, gn_b, scale)
    nc = _get_nc()
    in_maps = []
    for c in range(N_CORES):
        m = dict(consts)
        m["x"] = make_x_shard(x, c)
        in_maps.append(m)
    res = run_bass_kernel_spmd(nc, in_maps, core_ids=list(range(N_CORES)))
    return np.concatenate([res.results[c]["y"] for c in range(N_CORES)],
                          axis=0)
